# revision 44
# baseline (speedup 1.0000x reference)
"""Trainium2 Bass kernel for nn_CrossAttentionFuse.

Reference computation (per batch b):
    q = Q_tokens[b] @ Wq ; k = K_tokens[b] @ Wk ; v = V_tokens[b] @ Wv   (all [T, 1024])
    per head h (16 heads x 64): kw = k_h * weight_K[b][:, None]
    S = q_h @ kw_h.T / sqrt(64) ; P = softmax(S) ; o_h = P @ v_h
    y[b] = concat_h(o_h) @ Wo + bo

Sharding (8 cores): core c handles batch b = c//2, head-group g = c%2 (8 heads,
512 feats).  Each core computes a partial y (its 8 heads' contribution to the
output projection); host sums the two partials per batch and adds bo.

Per-core layout strategy (all activations kept feature-major, "transposed"):
  - inputs are host-transposed X^T [1024, T]
  - qT, kTw [feat, tok] tiles ([128, T] x4; feat tile f = heads 2f, 2f+1)
  - kTw = k^T * weight_K (folded during PSUM eviction; removes softmax scale)
  - 1/sqrt(64) folded into Wq on host
  - v token-major [tok, feat] (tiles [128, 512] x16)
  - scores computed transposed: S^T[k, q] = kTw_chunk.T-stationary @ qT
    (2 heads row-packed per matmul pair), softmax needs no max subtraction
    (|scores| < ~2 for this problem; exp cannot overflow)
  - P~ = exp(S^T) with ACT; per-key weights already in kTw
  - PV: out^T[d, q] += v_chunk-stationary @ P~_chunk, 2 heads col-packed
  - softmax denominators = ones-vector matmul over accumulated P~ partials,
    reciprocal on DVE, partition-broadcast via SWDGE DMA, applied during the
    PSUM eviction of out^T
  - final: y[tok, out] += attT_chunk-stationary @ Wo, evict + DMA
"""

import time
from contextlib import ExitStack
from dataclasses import dataclass

import numpy as np

import concourse.bass as bass
import concourse.tile as tile
from concourse import bacc, mybir
from concourse.bass_utils import run_bass_kernel_spmd

F32 = mybir.dt.float32
F32R = mybir.dt.float32r
BF16 = mybir.dt.bfloat16
EXP = mybir.ActivationFunctionType.Exp

N_CORES = 8
NUM_HEADS = 16
HD = 64


@dataclass(frozen=True)
class Cfg:
    D: int = 1024    # input embedding dim
    T: int = 2048    # tokens (Nq == Nk)
    F: int = 512     # projected feats per core (8 heads * 64)
    OUT: int = 1024  # Wo output dim
    QTW: int = 512   # q-tile width for attention

    @property
    def IC(self):    # input-dim chunks of 128
        return self.D // 128

    @property
    def KC(self):    # key chunks of 128
        return self.T // 128

    @property
    def NF(self):    # feat tiles of 128 (head pairs)
        return self.F // 128

    @property
    def NQT(self):   # q tiles
        return self.T // self.QTW

    @property
    def PW(self):    # projection pass width (token cols per psum pass)
        return min(self.T, 1024)

    @property
    def NPASS(self):
        return self.T // self.PW


FULL = Cfg()


def emit(ctx: ExitStack, tc, io: dict, cfg: Cfg, blevel: int = 4, do_c: bool = True):
    nc = tc.nc
    D, T, F, OUT, QTW = cfg.D, cfg.T, cfg.F, cfg.OUT, cfg.QTW
    IC, KC, NF, NQT = cfg.IC, cfg.KC, cfg.NF, cfg.NQT
    PJW = min(T, 512)        # projection psum pass width
    NPJ = T // PJW
    NO = max(OUT // 512, 1)  # out-dim chunks for final projection
    OCW = OUT // NO
    assert QTW <= 512 and OCW <= 512
    SPLIT_ADDS = False  # gpsimd TT too slow; congests bcast DMAs

    # ---------------- persistent pools / tiles ----------------
    qkv_pool = ctx.enter_context(tc.tile_pool(name="qkv", bufs=1))
    const_pool = ctx.enter_context(tc.tile_pool(name="const", bufs=1))
    x_pool = ctx.enter_context(tc.tile_pool(name="xchunk", bufs=min(IC, 8) + 4))
    w_pool = ctx.enter_context(tc.tile_pool(name="wproj", bufs=3))
    wv_pool = ctx.enter_context(tc.tile_pool(name="wvp", bufs=1))
    wkb_pool = ctx.enter_context(tc.tile_pool(name="wkbp", bufs=1))
    pj_psum = ctx.enter_context(tc.tile_pool(name="pjpsum", bufs=1, space="PSUM"))

    ones = const_pool.tile([128, 1], BF16, tag="ones")
    nc.vector.memset(ones, 1.0)

    qT = [qkv_pool.tile([128, T], BF16, tag=f"qT{f}", name=f"qT{f}") for f in range(NF)]
    kTw = [qkv_pool.tile([128, T], BF16, tag=f"kTw{f}", name=f"kTw{f}") for f in range(NF)]
    vtok = [qkv_pool.tile([128, F], BF16, tag=f"v{t}", name=f"v{t}") for t in range(KC)]

    # weight_K broadcast to all 128 partitions: [128, T]
    wkb = wkb_pool.tile([128, T], F32, tag="wkb")
    wk_ap = io["wkey"]
    wk_src = bass.AP(tensor=wk_ap.tensor, offset=wk_ap.offset,
                     ap=[[0, 128]] + list(wk_ap.ap))
    nc.gpsimd.dma_start(out=wkb[:], in_=wk_src)

    def load_x(name):
        # hold one full transposed input in SBUF as IC slabs [128, T]
        xs = []
        for i in range(IC):
            xt = x_pool.tile([128, T], BF16, tag="xchunk", name=f"x{name}{i}")
            nc.sync.dma_start(xt[:], io[name][i * 128:(i + 1) * 128, :])
            xs.append(xt)
        return xs

    # ------- q/k projections, emitted in half-passes so the PE detours are
    # ------- small enough to hide under the exp stream of the previous
    # ------- head-pair's attention loop -------
    def make_proj_passes(f, psum_pool, psum_tag):
        state = {}

        def prefetch(xname, wname, kind):
            def go():
                xs = load_x(xname)
                wt = w_pool.tile([128, IC, 128], BF16, tag="wproj",
                                 name=f"w{wname}{f}")
                nc.sync.dma_start(
                    wt[:],
                    io[wname][:, f * 128:(f + 1) * 128]
                    .rearrange("(i p) f -> p i f", p=128))
                state[kind] = (xs, wt)
            return go

        NP = 4 if IC >= 4 else IC  # pieces per pass (2 matmuls each at IC=8)

        def mk(dst, kind, pa, piece):
            def go():
                xs, wt = state[kind]
                psl = slice(pa * PJW, (pa + 1) * PJW)
                if piece == 0:
                    state["ps"] = psum_pool.tile([128, PJW], F32, tag=psum_tag,
                                                 name=f"pj{kind}{f}{pa}")
                ps = state["ps"]
                for i in range(piece * IC // NP, (piece + 1) * IC // NP):
                    nc.tensor.matmul(ps[:], wt[:, i, :], xs[i][:, psl],
                                     start=(i == 0), stop=(i == IC - 1))
                if piece == NP - 1:
                    if kind == "q":
                        nc.vector.tensor_copy(dst[:, psl], ps[:])
                    else:  # k: fold per-key weight during eviction
                        nc.vector.tensor_mul(dst[:, psl], ps[:], wkb[:, psl])
            return go

        halves = ([mk(qT[f], "q", pa, p) for pa in range(NPJ) for p in range(NP)]
                  + [mk(kTw[f], "k", pa, p) for pa in range(NPJ) for p in range(NP)])
        return prefetch("xq", "wq", "q"), prefetch("xk", "wk", "k"), halves

    # ---------------- stages B + C ----------------
    with ExitStack() as sb:
        att_pool = sb.enter_context(tc.tile_pool(name="attp", bufs=1))
        wo_pool = sb.enter_context(tc.tile_pool(name="wop", bufs=1))
        pexp_pool = sb.enter_context(tc.tile_pool(name="pexp", bufs=8))
        part_pool = sb.enter_context(tc.tile_pool(name="partials", bufs=4))
        inv_pool = sb.enter_context(tc.tile_pool(name="invp", bufs=2))
        scr_pool = sb.enter_context(tc.tile_pool(name="scrp", bufs=2, space="DRAM"))

        s_psum = sb.enter_context(tc.tile_pool(name="spsum", bufs=2, space="PSUM"))
        o_psum = sb.enter_context(tc.tile_pool(name="opsum", bufs=2, space="PSUM"))
        sm_psum = sb.enter_context(tc.tile_pool(name="smpsum", bufs=1, space="PSUM"))

        attT = [att_pool.tile([128, T], BF16, tag=f"attT{f}", name=f"attT{f}") for f in range(NF)]

        wo_sb = wo_pool.tile([128, F // 128, OUT], BF16, tag="wo")
        nc.sync.dma_start(
            wo_sb[:], io["wo"].rearrange("(i p) o -> p i o", p=128))

        # ---------------- v projection (token-major) ----------------
        # runs in the attention score psum slots (idle during the prelude)
        xs = load_x("xv")
        wv_sb = wv_pool.tile([128, IC, F], BF16, tag="wv")
        nc.sync.dma_start(wv_sb[:], io["wv"].rearrange("(i p) f -> p i f", p=128))
        for t in range(KC):
            ps = s_psum.tile([128, PJW], F32, tag="s", name=f"vps{t}")
            for i in range(IC):
                nc.tensor.matmul(ps[:, 0:F], xs[i][:, t * 128:(t + 1) * 128],
                                 wv_sb[:, i, 0:F],
                                 start=(i == 0), stop=(i == IC - 1))
            nc.vector.tensor_copy(vtok[t][:], ps[:, 0:F])

        def emit_pv(hp, c, pe_t, o_ps):
            # out^T accumulation, 2 heads col-packed into one bank
            nc.tensor.matmul(o_ps[0:64, :],
                             vtok[c][:, hp * 128:hp * 128 + 64],
                             pe_t[:, 0:QTW],
                             start=(c == 0), stop=(c == KC - 1),
                             skip_group_check=True)
            nc.tensor.matmul(o_ps[64:128, :],
                             vtok[c][:, hp * 128 + 64:hp * 128 + 128],
                             pe_t[:, QTW:2 * QTW],
                             start=(c == 0), stop=(c == KC - 1),
                             skip_group_check=True)

        def norm_tail_a1(hp, qt, o_ps, pab_t, pab2_t):
            # head-A denominator: one sums matmul + one reciprocal
            two = SPLIT_ADDS
            sm_ps = sm_psum.tile([128, QTW], F32, tag="sm")
            nc.tensor.matmul(sm_ps[0:1, :], ones[:], pab_t[:, 0:QTW],
                             start=True, stop=not two, skip_group_check=True)
            if two:
                nc.tensor.matmul(sm_ps[0:1, :], ones[:], pab2_t[:, 0:QTW],
                                 start=False, stop=True, skip_group_check=True)
            inv1 = inv_pool.tile([1, 2 * QTW], F32, tag="inv1")
            nc.vector.reciprocal(inv1[:, 0:QTW], sm_ps[0:1, :])
            return sm_ps, inv1

        def norm_tail_a2(hp, qt, o_ps, pab_t, pab2_t, sm_ps, inv1):
            # head-B denominator + start the partition-broadcast (DRAM
            # roundtrip; SBUF APs cannot have zero-step partition dims)
            two = SPLIT_ADDS
            nc.tensor.matmul(sm_ps[32:33, :], ones[:], pab_t[:, QTW:2 * QTW],
                             start=True, stop=not two, skip_group_check=True)
            if two:
                nc.tensor.matmul(sm_ps[32:33, :], ones[:],
                                 pab2_t[:, QTW:2 * QTW],
                                 start=False, stop=True, skip_group_check=True)
            nc.vector.reciprocal(inv1[:, QTW:2 * QTW], sm_ps[32:33, :])
            scr = scr_pool.tile([1, 2 * QTW], F32, tag="scr")
            nc.gpsimd.dma_start(scr[:], inv1[:])
            invb = inv_pool.tile([128, QTW], F32, tag="invb")
            scr_a = scr[:, 0:QTW]
            scr_b = scr[:, QTW:2 * QTW]
            nc.gpsimd.dma_start(
                out=invb[0:64, :],
                in_=bass.AP(tensor=scr_a.tensor, offset=scr_a.offset,
                            ap=[[0, 64], [1, QTW]]))
            nc.gpsimd.dma_start(
                out=invb[64:128, :],
                in_=bass.AP(tensor=scr_b.tensor, offset=scr_b.offset,
                            ap=[[0, 64], [1, QTW]]))
            return invb

        def norm_tail_b(hp, qt, o_ps, invb, half):
            # normalize + evict out^T into attT, one head per call (small DVE
            # detours keep the partial-add chain at the exp rate)
            qsl = slice(qt * QTW, (qt + 1) * QTW)
            rs = slice(0, 64) if half == 0 else slice(64, 128)
            nc.vector.tensor_mul(attT[hp][rs, qsl], o_ps[rs, :], invb[rs, :])

        y_pool = sb.enter_context(tc.tile_pool(name="yev", bufs=2))
        y_sbs = {}
        y_state = [0]

        def emit_y_half(t, n):
            # half an output-projection token tile; psum via the (idle) pj
            # pool, DVE eviction to keep ACT free for exp
            tsl = slice(t * 128, (t + 1) * 128)
            if n == 0:
                y_sbs[t] = y_pool.tile([128, OUT], F32, tag="ysb",
                                       name=f"ysb{t}")
            y_sb = y_sbs[t]
            y_ps = pj_psum.tile([128, OCW], F32, tag="pj", name=f"yps{t}_{n}")
            for f in range(NF):
                nc.tensor.matmul(
                    y_ps[:],
                    attT[f][:, tsl],
                    wo_sb[:, f, n * OCW:(n + 1) * OCW],
                    start=(f == 0), stop=(f == NF - 1))
            nc.vector.tensor_copy(y_sb[:, n * OCW:(n + 1) * OCW], y_ps[:])
            if n == NO - 1:
                nc.sync.dma_start(io["y"][tsl, :], y_sb[:])

        TA = min(2, KC - 1)   # chunk at which the previous tail's sums start
        TB = 11  # chunk at which the previous tail's mul runs
        pending = None
        pending_b = None

        def run_proj_serial(f, pool=None, tag=None):
            pq, pk, halves = make_proj_passes(f, pool or pj_psum, tag or "pj")
            pq()
            for h in halves[:len(halves) // 2]:
                h()
            pk()
            for h in halves[len(halves) // 2:]:
                h()

        run_proj_serial(0, o_psum, "o")
        if blevel == 0:
            for f in range(1, NF):
                run_proj_serial(f)
            return
        for hp in range(NF):          # head pair = feat tile
            if hp + 1 < NF:
                pf_q, pf_k, next_passes = make_proj_passes(hp + 1, pj_psum, "pj")
            else:
                pf_q = pf_k = None
                next_passes = []
            npi = 0
            for qt in range(NQT):
                qsl = slice(qt * QTW, (qt + 1) * QTW)
                o_ps = o_psum.tile([128, QTW], F32, tag="o")
                pab_t = part_pool.tile([128, 2 * QTW], BF16, tag="pab")
                pab2_t = (part_pool.tile([128, 2 * QTW], BF16, tag="pab2",
                                         name="pab2_t")
                          if SPLIT_ADDS else None)
                pv_queue = []
                for c in range(KC):
                    csl = slice(c * 128, (c + 1) * 128)
                    s_ps = s_psum.tile([128, 2 * QTW], F32, tag="s")
                    # S^T chunk, head A (rows 0:64) and B (64:128) row-packed
                    nc.tensor.matmul(s_ps[:, 0:QTW],
                                     kTw[hp][0:64, csl], qT[hp][0:64, qsl],
                                     start=True, stop=True)
                    nc.tensor.matmul(s_ps[:, QTW:2 * QTW],
                                     kTw[hp][64:128, csl], qT[hp][64:128, qsl],
                                     start=True, stop=True)
                    if blevel >= 2:
                        # P~ = exp(S^T), both heads in one ACT op
                        pe_t = pexp_pool.tile([128, 2 * QTW], BF16, tag="pe")
                        nc.scalar.activation(pe_t[:], s_ps[:], EXP)
                        # running partial sums (softmax denominators),
                        # split 3:1 across DVE and GPSIMD (independent
                        # accumulators, merged in the sums matmuls)
                        if blevel >= 4:
                            if SPLIT_ADDS and c % 4 == 3:
                                if c == 3:
                                    nc.gpsimd.tensor_copy(pab2_t[:], pe_t[:])
                                else:
                                    nc.gpsimd.tensor_add(pab2_t[:], pab2_t[:],
                                                         pe_t[:])
                            elif c == 0:
                                nc.vector.tensor_copy(pab_t[:], pe_t[:])
                            else:
                                nc.vector.tensor_add(pab_t[:], pab_t[:],
                                                     pe_t[:])
                        if blevel >= 3:
                            # PV deferred and emitted in pairs on odd chunks:
                            # the PE stream sees QK,QK,PV,PV per 2 chunks,
                            # halving the row-conflicting LDW transitions,
                            # and never stalls on exp with QK work queued
                            pv_queue.append((c, pe_t))
                            while len(pv_queue) > 1:
                                pc, ppe = pv_queue.pop(0)
                                emit_pv(hp, pc, ppe, o_ps)
                    if pf_q is not None and qt == 0 and c == 0:
                        pf_q()
                    if pf_k is not None and qt == 1 and c == 8:
                        pf_k()
                    if KC >= 8 and c == TA and pending is not None:
                        st = norm_tail_a1(*pending)
                        pending = pending + st
                    if KC >= 8 and c == TA + 3 and pending is not None:
                        invb = norm_tail_a2(*pending)
                        pending_b = (pending[0], pending[1], pending[2], invb)
                        pending = None
                    if KC >= 8 and c == TB and pending_b is not None:
                        norm_tail_b(*pending_b, 0)
                    if KC >= 8 and c == TB + 2 and pending_b is not None:
                        norm_tail_b(*pending_b, 1)
                        pending_b = None
                    slots = ((3, 4, 6, 7, 8, 10, 14) if qt == 0
                             else (1, 3, 4, 6, 7, 8, 10, 14))
                    if c in slots and npi < len(next_passes):
                        next_passes[npi]()
                        npi += 1
                    if do_c and blevel >= 4 and hp == NF - 1:
                        ready_t = ((qt - 1) if c >= TB + 1 else (qt - 2)) \
                            * (QTW // 128)
                        slots_y = ((11, 13, 15) if qt == 1
                                   else (1, 3, 5, 7, 9, 11, 13, 15))
                        if (qt >= 1 and c in slots_y
                                and y_state[0] < NO * max(ready_t, 0)):
                            emit_y_half(y_state[0] // NO, y_state[0] % NO)
                            y_state[0] += 1
                for pc, ppe in pv_queue:
                    emit_pv(hp, pc, ppe, o_ps)
                pv_queue = []
                if blevel >= 4:
                    if KC < 8 and pending is not None:
                        # small-KC fallback: run the whole tail serially
                        st = norm_tail_a1(*pending)
                        invb = norm_tail_a2(*(pending + st))
                        norm_tail_b(pending[0], pending[1], pending[2], invb, 0)
                        norm_tail_b(pending[0], pending[1], pending[2], invb, 1)
                    pending = (hp, qt, o_ps, pab_t, pab2_t)
            # any projection passes not yet injected (shouldn't happen for
            # NQT*2 >= 2*NPJ, but keep correct regardless)
            while npi < len(next_passes):
                next_passes[npi]()
                npi += 1
        if pending is not None:
            if len(pending) == 5:
                pending = pending + norm_tail_a1(*pending)
            invb = norm_tail_a2(*pending)
            pending_b = (pending[0], pending[1], pending[2], invb)
        if pending_b is not None:
            norm_tail_b(*pending_b, 0)
            norm_tail_b(*pending_b, 1)

        if not do_c or blevel < 4:
            return
        # ---------------- stage C: remaining output tiles ----------------
        while y_state[0] < NO * KC:
            emit_y_half(y_state[0] // NO, y_state[0] % NO)
            y_state[0] += 1


def build_program(cfg: Cfg = FULL, n_iters: int = 1, blevel: int = 4, do_c: bool = True):
    nc = bacc.Bacc("TRN2", target_bir_lowering=False, debug=False,
                   num_devices=N_CORES)
    io = {
        "xq": nc.dram_tensor("xq", [cfg.D, cfg.T], BF16, kind="ExternalInput").ap(),
        "xk": nc.dram_tensor("xk", [cfg.D, cfg.T], BF16, kind="ExternalInput").ap(),
        "xv": nc.dram_tensor("xv", [cfg.D, cfg.T], BF16, kind="ExternalInput").ap(),
        "wq": nc.dram_tensor("wq", [cfg.D, cfg.F], BF16, kind="ExternalInput").ap(),
        "wk": nc.dram_tensor("wk", [cfg.D, cfg.F], BF16, kind="ExternalInput").ap(),
        "wv": nc.dram_tensor("wv", [cfg.D, cfg.F], BF16, kind="ExternalInput").ap(),
        "wo": nc.dram_tensor("wo", [cfg.F, cfg.OUT], BF16, kind="ExternalInput").ap(),
        "wkey": nc.dram_tensor("wkey", [cfg.T], F32, kind="ExternalInput").ap(),
        "y": nc.dram_tensor("y", [cfg.T, cfg.OUT], F32, kind="ExternalOutput").ap(),
    }
    with tile.TileContext(nc) as tc:
        with ExitStack() as ctx:
            if n_iters == 1:
                emit(ctx, tc, io, cfg, blevel, do_c)
            else:
                with tc.For_i(0, n_iters, 1):
                    emit(ctx, tc, io, cfg, blevel, do_c)
    nc.compile()
    return nc


def shard_inputs(Q_tokens, K_tokens, V_tokens, weight_K, Wq, Wk, Wv, Wo, bo):
    """Build the 8 per-core input maps (full -> sharded, host side)."""
    import ml_dtypes
    bf = ml_dtypes.bfloat16
    scale = 1.0 / np.sqrt(np.float32(HD))
    Wq_s = (np.asarray(Wq, np.float32) * scale)
    in_maps = []
    qt = np.ascontiguousarray(np.asarray(Q_tokens, np.float32).transpose(0, 2, 1)).astype(bf)
    kt = np.ascontiguousarray(np.asarray(K_tokens, np.float32).transpose(0, 2, 1)).astype(bf)
    vt = np.ascontiguousarray(np.asarray(V_tokens, np.float32).transpose(0, 2, 1)).astype(bf)
    for c in range(N_CORES):
        b, g = divmod(c, 2)
        fs = slice(g * 512, (g + 1) * 512)
        in_maps.append({
            "xq": qt[b],
            "xk": kt[b],
            "xv": vt[b],
            "wq": np.ascontiguousarray(Wq_s[:, fs]).astype(bf),
            "wk": np.ascontiguousarray(np.asarray(Wk, np.float32)[:, fs]).astype(bf),
            "wv": np.ascontiguousarray(np.asarray(Wv, np.float32)[:, fs]).astype(bf),
            "wo": np.ascontiguousarray(np.asarray(Wo, np.float32)[fs, :]).astype(bf),
            "wkey": np.ascontiguousarray(np.asarray(weight_K, np.float32)[b]),
        })
    return in_maps


_PROGRAM_CACHE: dict = {}


def _get_program(n_iters: int = 1, blevel: int = 4, do_c: bool = True):
    key = (n_iters, blevel, do_c)
    if key not in _PROGRAM_CACHE:
        _PROGRAM_CACHE[key] = build_program(FULL, n_iters, blevel, do_c)
    return _PROGRAM_CACHE[key]


def run_sharded(in_maps, n_iters: int = 1):
    nc = _get_program(n_iters)
    t0 = time.time()
    res = run_bass_kernel_spmd(nc, in_maps, core_ids=list(range(N_CORES)))
    wall = time.time() - t0
    return res, wall


def kernel(Q_tokens, K_tokens, V_tokens, weight_K, Wq, Wk, Wv, Wo, bo):
    in_maps = shard_inputs(Q_tokens, K_tokens, V_tokens, weight_K,
                           Wq, Wk, Wv, Wo, bo)
    res, _ = run_sharded(in_maps)
    B = np.asarray(Q_tokens).shape[0]
    T, OUT = FULL.T, FULL.OUT
    out = np.empty((B, T, OUT), np.float32)
    bo = np.asarray(bo, np.float32)
    for b in range(B):
        out[b] = res.results[2 * b]["y"] + res.results[2 * b + 1]["y"] + bo
    return out


# revision 45
# speedup vs baseline: 1.0584x; 1.0584x over previous
"""Trainium2 Bass kernel for nn_CrossAttentionFuse.

Reference computation (per batch b):
    q = Q_tokens[b] @ Wq ; k = K_tokens[b] @ Wk ; v = V_tokens[b] @ Wv   (all [T, 1024])
    per head h (16 heads x 64): kw = k_h * weight_K[b][:, None]
    S = q_h @ kw_h.T / sqrt(64) ; P = softmax(S) ; o_h = P @ v_h
    y[b] = concat_h(o_h) @ Wo + bo

Sharding (8 cores): core c handles batch b = c//2, head-group g = c%2 (8 heads,
512 feats).  Each core computes a partial y (its 8 heads' contribution to the
output projection); host sums the two partials per batch and adds bo.

Per-core layout strategy (all activations kept feature-major, "transposed"):
  - inputs are host-transposed X^T [1024, T]
  - qT, kTw [feat, tok] tiles ([128, T] x4; feat tile f = heads 2f, 2f+1)
  - kTw = k^T * weight_K (folded during PSUM eviction; removes softmax scale)
  - 1/sqrt(64) folded into Wq on host
  - v token-major [tok, feat] (tiles [128, 512] x16)
  - scores computed transposed: S^T[k, q] = kTw_chunk.T-stationary @ qT
    (2 heads row-packed per matmul pair), softmax needs no max subtraction
    (|scores| < ~2 for this problem; exp cannot overflow)
  - P~ = exp(S^T) with ACT; per-key weights already in kTw
  - PV: out^T[d, q] += v_chunk-stationary @ P~_chunk, 2 heads col-packed
  - softmax denominators = ones-vector matmul over accumulated P~ partials,
    reciprocal on DVE, partition-broadcast via SWDGE DMA, applied during the
    PSUM eviction of out^T
  - final: y[tok, out] += attT_chunk-stationary @ Wo, evict + DMA
"""

import time
from contextlib import ExitStack
from dataclasses import dataclass

import numpy as np

import concourse.bass as bass
import concourse.tile as tile
from concourse import bacc, mybir
from concourse.bass_utils import run_bass_kernel_spmd

F32 = mybir.dt.float32
F32R = mybir.dt.float32r
BF16 = mybir.dt.bfloat16
EXP = mybir.ActivationFunctionType.Exp

N_CORES = 8
NUM_HEADS = 16
HD = 64


@dataclass(frozen=True)
class Cfg:
    D: int = 1024    # input embedding dim
    T: int = 2048    # tokens (Nq == Nk)
    F: int = 512     # projected feats per core (8 heads * 64)
    OUT: int = 1024  # Wo output dim
    QTW: int = 512   # q-tile width for attention

    @property
    def IC(self):    # input-dim chunks of 128
        return self.D // 128

    @property
    def KC(self):    # key chunks of 128
        return self.T // 128

    @property
    def NF(self):    # feat tiles of 128 (head pairs)
        return self.F // 128

    @property
    def NQT(self):   # q tiles
        return self.T // self.QTW

    @property
    def PW(self):    # projection pass width (token cols per psum pass)
        return min(self.T, 1024)

    @property
    def NPASS(self):
        return self.T // self.PW


FULL = Cfg()


def emit(ctx: ExitStack, tc, io: dict, cfg: Cfg, blevel: int = 4, do_c: bool = True):
    nc = tc.nc
    D, T, F, OUT, QTW = cfg.D, cfg.T, cfg.F, cfg.OUT, cfg.QTW
    IC, KC, NF, NQT = cfg.IC, cfg.KC, cfg.NF, cfg.NQT
    PJW = min(T, 512)        # projection psum pass width
    NPJ = T // PJW
    NO = max(OUT // 512, 1)  # out-dim chunks for final projection
    OCW = OUT // NO
    assert QTW <= 512 and OCW <= 512
    SPLIT_ADDS = False  # gpsimd TT too slow; congests bcast DMAs

    # ---------------- persistent pools / tiles ----------------
    qkv_pool = ctx.enter_context(tc.tile_pool(name="qkv", bufs=1))
    const_pool = ctx.enter_context(tc.tile_pool(name="const", bufs=1))
    x_pool = ctx.enter_context(tc.tile_pool(name="xchunk", bufs=min(IC, 8) + 4))
    w_pool = ctx.enter_context(tc.tile_pool(name="wproj", bufs=3))
    wv_pool = ctx.enter_context(tc.tile_pool(name="wvp", bufs=1))
    wkb_pool = ctx.enter_context(tc.tile_pool(name="wkbp", bufs=1))
    pj_psum = ctx.enter_context(tc.tile_pool(name="pjpsum", bufs=1, space="PSUM"))

    ones = const_pool.tile([128, 1], BF16, tag="ones")
    nc.vector.memset(ones, 1.0)

    qT = [qkv_pool.tile([128, T], BF16, tag=f"qT{f}", name=f"qT{f}") for f in range(NF)]
    kTw = [qkv_pool.tile([128, T], BF16, tag=f"kTw{f}", name=f"kTw{f}") for f in range(NF)]
    vtok = [qkv_pool.tile([128, F], BF16, tag=f"v{t}", name=f"v{t}") for t in range(KC)]

    # weight_K broadcast to all 128 partitions: [128, T]
    wkb = wkb_pool.tile([128, T], F32, tag="wkb")
    wk_ap = io["wkey"]
    wk_src = bass.AP(tensor=wk_ap.tensor, offset=wk_ap.offset,
                     ap=[[0, 128]] + list(wk_ap.ap))
    nc.gpsimd.dma_start(out=wkb[:], in_=wk_src)

    def load_x(name):
        # hold one full transposed input in SBUF as IC slabs [128, T]
        xs = []
        for i in range(IC):
            xt = x_pool.tile([128, T], BF16, tag="xchunk", name=f"x{name}{i}")
            nc.sync.dma_start(xt[:], io[name][i * 128:(i + 1) * 128, :])
            xs.append(xt)
        return xs

    # ------- q/k projections, emitted in half-passes so the PE detours are
    # ------- small enough to hide under the exp stream of the previous
    # ------- head-pair's attention loop -------
    def make_proj_passes(f, psum_pool, psum_tag):
        state = {}

        def prefetch(xname, wname, kind):
            def go():
                xs = load_x(xname)
                wt = w_pool.tile([128, IC, 128], BF16, tag="wproj",
                                 name=f"w{wname}{f}")
                nc.sync.dma_start(
                    wt[:],
                    io[wname][:, f * 128:(f + 1) * 128]
                    .rearrange("(i p) f -> p i f", p=128))
                state[kind] = (xs, wt)
            return go

        NP = 2 if IC >= 2 else 1  # pieces per pass (4 matmuls each at IC=8)

        def mk(dst, kind, pa, piece):
            def go():
                xs, wt = state[kind]
                psl = slice(pa * PJW, (pa + 1) * PJW)
                if piece == 0:
                    state["ps"] = psum_pool.tile([128, PJW], F32, tag=psum_tag,
                                                 name=f"pj{kind}{f}{pa}")
                ps = state["ps"]
                for i in range(piece * IC // NP, (piece + 1) * IC // NP):
                    nc.tensor.matmul(ps[:], wt[:, i, :], xs[i][:, psl],
                                     start=(i == 0), stop=(i == IC - 1))
                if piece == NP - 1:
                    if kind == "q":
                        nc.vector.tensor_copy(dst[:, psl], ps[:])
                    else:  # k: fold per-key weight during eviction
                        nc.vector.tensor_mul(dst[:, psl], ps[:], wkb[:, psl])
            return go

        halves = ([mk(qT[f], "q", pa, p) for pa in range(NPJ) for p in range(NP)]
                  + [mk(kTw[f], "k", pa, p) for pa in range(NPJ) for p in range(NP)])
        return prefetch("xq", "wq", "q"), prefetch("xk", "wk", "k"), halves

    # ---------------- stages B + C ----------------
    with ExitStack() as sb:
        att_pool = sb.enter_context(tc.tile_pool(name="attp", bufs=1))
        wo_pool = sb.enter_context(tc.tile_pool(name="wop", bufs=1))
        pexp_pool = sb.enter_context(tc.tile_pool(name="pexp", bufs=8))
        part_pool = sb.enter_context(tc.tile_pool(name="partials", bufs=4))
        inv_pool = sb.enter_context(tc.tile_pool(name="invp", bufs=2))
        scr_pool = sb.enter_context(tc.tile_pool(name="scrp", bufs=2, space="DRAM"))

        s_psum = sb.enter_context(tc.tile_pool(name="spsum", bufs=2, space="PSUM"))
        o_psum = sb.enter_context(tc.tile_pool(name="opsum", bufs=2, space="PSUM"))
        sm_psum = sb.enter_context(tc.tile_pool(name="smpsum", bufs=1, space="PSUM"))

        attT = [att_pool.tile([128, T], BF16, tag=f"attT{f}", name=f"attT{f}") for f in range(NF)]

        wo_sb = wo_pool.tile([128, F // 128, OUT], BF16, tag="wo")
        nc.sync.dma_start(
            wo_sb[:], io["wo"].rearrange("(i p) o -> p i o", p=128))

        # ---------------- v projection (token-major) ----------------
        # runs in the attention score psum slots (idle during the prelude)
        xs = load_x("xv")
        wv_sb = wv_pool.tile([128, IC, F], BF16, tag="wv")
        nc.sync.dma_start(wv_sb[:], io["wv"].rearrange("(i p) f -> p i f", p=128))
        for t in range(KC):
            ps = s_psum.tile([128, PJW], F32, tag="s", name=f"vps{t}")
            for i in range(IC):
                nc.tensor.matmul(ps[:, 0:F], xs[i][:, t * 128:(t + 1) * 128],
                                 wv_sb[:, i, 0:F],
                                 start=(i == 0), stop=(i == IC - 1))
            nc.vector.tensor_copy(vtok[t][:], ps[:, 0:F])

        def emit_pv(hp, c, pe_t, o_ps):
            # out^T accumulation, 2 heads col-packed into one bank
            nc.tensor.matmul(o_ps[0:64, :],
                             vtok[c][:, hp * 128:hp * 128 + 64],
                             pe_t[:, 0:QTW],
                             start=(c == 0), stop=(c == KC - 1),
                             skip_group_check=True)
            nc.tensor.matmul(o_ps[64:128, :],
                             vtok[c][:, hp * 128 + 64:hp * 128 + 128],
                             pe_t[:, QTW:2 * QTW],
                             start=(c == 0), stop=(c == KC - 1),
                             skip_group_check=True)

        def norm_tail_a1(hp, qt, o_ps, pab_t, pab2_t):
            # head-A denominator: one sums matmul + one reciprocal
            two = SPLIT_ADDS
            sm_ps = sm_psum.tile([128, QTW], F32, tag="sm")
            nc.tensor.matmul(sm_ps[0:1, :], ones[:], pab_t[:, 0:QTW],
                             start=True, stop=not two, skip_group_check=True)
            if two:
                nc.tensor.matmul(sm_ps[0:1, :], ones[:], pab2_t[:, 0:QTW],
                                 start=False, stop=True, skip_group_check=True)
            inv1 = inv_pool.tile([1, 2 * QTW], F32, tag="inv1")
            nc.vector.reciprocal(inv1[:, 0:QTW], sm_ps[0:1, :])
            return sm_ps, inv1

        def norm_tail_a2(hp, qt, o_ps, pab_t, pab2_t, sm_ps, inv1):
            # head-B denominator + start the partition-broadcast (DRAM
            # roundtrip; SBUF APs cannot have zero-step partition dims)
            two = SPLIT_ADDS
            nc.tensor.matmul(sm_ps[32:33, :], ones[:], pab_t[:, QTW:2 * QTW],
                             start=True, stop=not two, skip_group_check=True)
            if two:
                nc.tensor.matmul(sm_ps[32:33, :], ones[:],
                                 pab2_t[:, QTW:2 * QTW],
                                 start=False, stop=True, skip_group_check=True)
            nc.vector.reciprocal(inv1[:, QTW:2 * QTW], sm_ps[32:33, :])
            scr = scr_pool.tile([1, 2 * QTW], F32, tag="scr")
            nc.gpsimd.dma_start(scr[:], inv1[:])
            invb = inv_pool.tile([128, QTW], F32, tag="invb")
            scr_a = scr[:, 0:QTW]
            scr_b = scr[:, QTW:2 * QTW]
            nc.gpsimd.dma_start(
                out=invb[0:64, :],
                in_=bass.AP(tensor=scr_a.tensor, offset=scr_a.offset,
                            ap=[[0, 64], [1, QTW]]))
            nc.gpsimd.dma_start(
                out=invb[64:128, :],
                in_=bass.AP(tensor=scr_b.tensor, offset=scr_b.offset,
                            ap=[[0, 64], [1, QTW]]))
            return invb

        def norm_tail_b(hp, qt, o_ps, invb, half):
            # normalize + evict out^T into attT, one head per call (small DVE
            # detours keep the partial-add chain at the exp rate)
            qsl = slice(qt * QTW, (qt + 1) * QTW)
            rs = slice(0, 64) if half == 0 else slice(64, 128)
            nc.vector.tensor_mul(attT[hp][rs, qsl], o_ps[rs, :], invb[rs, :])

        y_pool = sb.enter_context(tc.tile_pool(name="yev", bufs=2))
        y_sbs = {}
        y_state = [0]

        def emit_y_half(t, n):
            # half an output-projection token tile; psum via the (idle) pj
            # pool, DVE eviction to keep ACT free for exp
            tsl = slice(t * 128, (t + 1) * 128)
            if n == 0:
                y_sbs[t] = y_pool.tile([128, OUT], F32, tag="ysb",
                                       name=f"ysb{t}")
            y_sb = y_sbs[t]
            y_ps = pj_psum.tile([128, OCW], F32, tag="pj", name=f"yps{t}_{n}")
            for f in range(NF):
                nc.tensor.matmul(
                    y_ps[:],
                    attT[f][:, tsl],
                    wo_sb[:, f, n * OCW:(n + 1) * OCW],
                    start=(f == 0), stop=(f == NF - 1))
            nc.vector.tensor_copy(y_sb[:, n * OCW:(n + 1) * OCW], y_ps[:])
            if n == NO - 1:
                nc.sync.dma_start(io["y"][tsl, :], y_sb[:])

        TA = min(2, KC - 1)   # chunk at which the previous tail's sums start
        TB = 11  # chunk at which the previous tail's mul runs
        pending = None
        pending_b = None

        def run_proj_serial(f, pool=None, tag=None):
            pq, pk, halves = make_proj_passes(f, pool or pj_psum, tag or "pj")
            pq()
            for h in halves[:len(halves) // 2]:
                h()
            pk()
            for h in halves[len(halves) // 2:]:
                h()

        run_proj_serial(0, o_psum, "o")
        if blevel == 0:
            for f in range(1, NF):
                run_proj_serial(f)
            return
        for hp in range(NF):          # head pair = feat tile
            if hp + 1 < NF:
                pf_q, pf_k, next_passes = make_proj_passes(hp + 1, pj_psum, "pj")
            else:
                pf_q = pf_k = None
                next_passes = []
            npi = 0
            for qt in range(NQT):
                qsl = slice(qt * QTW, (qt + 1) * QTW)
                o_ps = o_psum.tile([128, QTW], F32, tag="o")
                pab_t = part_pool.tile([128, 2 * QTW], BF16, tag="pab")
                pab2_t = (part_pool.tile([128, 2 * QTW], BF16, tag="pab2",
                                         name="pab2_t")
                          if SPLIT_ADDS else None)
                pv_queue = []
                for c in range(KC):
                    csl = slice(c * 128, (c + 1) * 128)
                    s_ps = s_psum.tile([128, 2 * QTW], F32, tag="s")
                    # S^T chunk, head A (rows 0:64) and B (64:128) row-packed
                    nc.tensor.matmul(s_ps[:, 0:QTW],
                                     kTw[hp][0:64, csl], qT[hp][0:64, qsl],
                                     start=True, stop=True)
                    nc.tensor.matmul(s_ps[:, QTW:2 * QTW],
                                     kTw[hp][64:128, csl], qT[hp][64:128, qsl],
                                     start=True, stop=True)
                    if blevel >= 2:
                        # P~ = exp(S^T), both heads in one ACT op
                        pe_t = pexp_pool.tile([128, 2 * QTW], BF16, tag="pe")
                        nc.scalar.activation(pe_t[:], s_ps[:], EXP)
                        # running partial sums (softmax denominators),
                        # split 3:1 across DVE and GPSIMD (independent
                        # accumulators, merged in the sums matmuls)
                        if blevel >= 4:
                            if SPLIT_ADDS and c % 4 == 3:
                                if c == 3:
                                    nc.gpsimd.tensor_copy(pab2_t[:], pe_t[:])
                                else:
                                    nc.gpsimd.tensor_add(pab2_t[:], pab2_t[:],
                                                         pe_t[:])
                            elif c == 0:
                                nc.vector.tensor_copy(pab_t[:], pe_t[:])
                            else:
                                nc.vector.tensor_add(pab_t[:], pab_t[:],
                                                     pe_t[:])
                        if blevel >= 3:
                            # PV deferred and emitted in pairs on odd chunks:
                            # the PE stream sees QK,QK,PV,PV per 2 chunks,
                            # halving the row-conflicting LDW transitions,
                            # and never stalls on exp with QK work queued
                            pv_queue.append((c, pe_t))
                            while len(pv_queue) > 1:
                                pc, ppe = pv_queue.pop(0)
                                emit_pv(hp, pc, ppe, o_ps)
                    if pf_q is not None and qt == 0 and c == 0:
                        pf_q()
                    if pf_k is not None and qt == 1 and c == 8:
                        pf_k()
                    if KC >= 8 and c == TA and pending is not None:
                        st = norm_tail_a1(*pending)
                        pending = pending + st
                    if KC >= 8 and c == TA + 3 and pending is not None:
                        invb = norm_tail_a2(*pending)
                        pending_b = (pending[0], pending[1], pending[2], invb)
                        pending = None
                    if KC >= 8 and c == TB and pending_b is not None:
                        norm_tail_b(*pending_b, 0)
                    if KC >= 8 and c == TB + 2 and pending_b is not None:
                        norm_tail_b(*pending_b, 1)
                        pending_b = None
                    slots = (8, 12, 15) if qt == 0 else (4, 8, 12, 15)
                    if c in slots and npi < len(next_passes):
                        next_passes[npi]()
                        npi += 1
                    if do_c and blevel >= 4 and hp == NF - 1:
                        ready_t = ((qt - 1) if c >= TB + 1 else (qt - 2)) \
                            * (QTW // 128)
                        slots_y = ((11, 13, 15) if qt == 1
                                   else (1, 3, 5, 7, 9, 11, 13, 15))
                        if (qt >= 1 and c in slots_y
                                and y_state[0] < NO * max(ready_t, 0)):
                            emit_y_half(y_state[0] // NO, y_state[0] % NO)
                            y_state[0] += 1
                for pc, ppe in pv_queue:
                    emit_pv(hp, pc, ppe, o_ps)
                pv_queue = []
                if blevel >= 4:
                    if KC < 8 and pending is not None:
                        # small-KC fallback: run the whole tail serially
                        st = norm_tail_a1(*pending)
                        invb = norm_tail_a2(*(pending + st))
                        norm_tail_b(pending[0], pending[1], pending[2], invb, 0)
                        norm_tail_b(pending[0], pending[1], pending[2], invb, 1)
                    pending = (hp, qt, o_ps, pab_t, pab2_t)
            # any projection passes not yet injected (shouldn't happen for
            # NQT*2 >= 2*NPJ, but keep correct regardless)
            while npi < len(next_passes):
                next_passes[npi]()
                npi += 1
        if pending is not None:
            if len(pending) == 5:
                pending = pending + norm_tail_a1(*pending)
            invb = norm_tail_a2(*pending)
            pending_b = (pending[0], pending[1], pending[2], invb)
        if pending_b is not None:
            norm_tail_b(*pending_b, 0)
            norm_tail_b(*pending_b, 1)

        if not do_c or blevel < 4:
            return
        # ---------------- stage C: remaining output tiles ----------------
        while y_state[0] < NO * KC:
            emit_y_half(y_state[0] // NO, y_state[0] % NO)
            y_state[0] += 1


def build_program(cfg: Cfg = FULL, n_iters: int = 1, blevel: int = 4, do_c: bool = True):
    nc = bacc.Bacc("TRN2", target_bir_lowering=False, debug=False,
                   num_devices=N_CORES)
    io = {
        "xq": nc.dram_tensor("xq", [cfg.D, cfg.T], BF16, kind="ExternalInput").ap(),
        "xk": nc.dram_tensor("xk", [cfg.D, cfg.T], BF16, kind="ExternalInput").ap(),
        "xv": nc.dram_tensor("xv", [cfg.D, cfg.T], BF16, kind="ExternalInput").ap(),
        "wq": nc.dram_tensor("wq", [cfg.D, cfg.F], BF16, kind="ExternalInput").ap(),
        "wk": nc.dram_tensor("wk", [cfg.D, cfg.F], BF16, kind="ExternalInput").ap(),
        "wv": nc.dram_tensor("wv", [cfg.D, cfg.F], BF16, kind="ExternalInput").ap(),
        "wo": nc.dram_tensor("wo", [cfg.F, cfg.OUT], BF16, kind="ExternalInput").ap(),
        "wkey": nc.dram_tensor("wkey", [cfg.T], F32, kind="ExternalInput").ap(),
        "y": nc.dram_tensor("y", [cfg.T, cfg.OUT], F32, kind="ExternalOutput").ap(),
    }
    with tile.TileContext(nc) as tc:
        with ExitStack() as ctx:
            if n_iters == 1:
                emit(ctx, tc, io, cfg, blevel, do_c)
            else:
                with tc.For_i(0, n_iters, 1):
                    emit(ctx, tc, io, cfg, blevel, do_c)
    nc.compile()
    return nc


def shard_inputs(Q_tokens, K_tokens, V_tokens, weight_K, Wq, Wk, Wv, Wo, bo):
    """Build the 8 per-core input maps (full -> sharded, host side)."""
    import ml_dtypes
    bf = ml_dtypes.bfloat16
    scale = 1.0 / np.sqrt(np.float32(HD))
    Wq_s = (np.asarray(Wq, np.float32) * scale)
    in_maps = []
    qt = np.ascontiguousarray(np.asarray(Q_tokens, np.float32).transpose(0, 2, 1)).astype(bf)
    kt = np.ascontiguousarray(np.asarray(K_tokens, np.float32).transpose(0, 2, 1)).astype(bf)
    vt = np.ascontiguousarray(np.asarray(V_tokens, np.float32).transpose(0, 2, 1)).astype(bf)
    for c in range(N_CORES):
        b, g = divmod(c, 2)
        fs = slice(g * 512, (g + 1) * 512)
        in_maps.append({
            "xq": qt[b],
            "xk": kt[b],
            "xv": vt[b],
            "wq": np.ascontiguousarray(Wq_s[:, fs]).astype(bf),
            "wk": np.ascontiguousarray(np.asarray(Wk, np.float32)[:, fs]).astype(bf),
            "wv": np.ascontiguousarray(np.asarray(Wv, np.float32)[:, fs]).astype(bf),
            "wo": np.ascontiguousarray(np.asarray(Wo, np.float32)[fs, :]).astype(bf),
            "wkey": np.ascontiguousarray(np.asarray(weight_K, np.float32)[b]),
        })
    return in_maps


_PROGRAM_CACHE: dict = {}


def _get_program(n_iters: int = 1, blevel: int = 4, do_c: bool = True):
    key = (n_iters, blevel, do_c)
    if key not in _PROGRAM_CACHE:
        _PROGRAM_CACHE[key] = build_program(FULL, n_iters, blevel, do_c)
    return _PROGRAM_CACHE[key]


def run_sharded(in_maps, n_iters: int = 1):
    nc = _get_program(n_iters)
    t0 = time.time()
    res = run_bass_kernel_spmd(nc, in_maps, core_ids=list(range(N_CORES)))
    wall = time.time() - t0
    return res, wall


def kernel(Q_tokens, K_tokens, V_tokens, weight_K, Wq, Wk, Wv, Wo, bo):
    in_maps = shard_inputs(Q_tokens, K_tokens, V_tokens, weight_K,
                           Wq, Wk, Wv, Wo, bo)
    res, _ = run_sharded(in_maps)
    B = np.asarray(Q_tokens).shape[0]
    T, OUT = FULL.T, FULL.OUT
    out = np.empty((B, T, OUT), np.float32)
    bo = np.asarray(bo, np.float32)
    for b in range(B):
        out[b] = res.results[2 * b]["y"] + res.results[2 * b + 1]["y"] + bo
    return out


# revision 46
# speedup vs baseline: 1.0665x; 1.0077x over previous
"""Trainium2 Bass kernel for nn_CrossAttentionFuse.

Reference computation (per batch b):
    q = Q_tokens[b] @ Wq ; k = K_tokens[b] @ Wk ; v = V_tokens[b] @ Wv   (all [T, 1024])
    per head h (16 heads x 64): kw = k_h * weight_K[b][:, None]
    S = q_h @ kw_h.T / sqrt(64) ; P = softmax(S) ; o_h = P @ v_h
    y[b] = concat_h(o_h) @ Wo + bo

Sharding (8 cores): core c handles batch b = c//2, head-group g = c%2 (8 heads,
512 feats).  Each core computes a partial y (its 8 heads' contribution to the
output projection); host sums the two partials per batch and adds bo.

Per-core layout strategy (all activations kept feature-major, "transposed"):
  - inputs are host-transposed X^T [1024, T]
  - qT, kTw [feat, tok] tiles ([128, T] x4; feat tile f = heads 2f, 2f+1)
  - kTw = k^T * weight_K (folded during PSUM eviction; removes softmax scale)
  - 1/sqrt(64) folded into Wq on host
  - v token-major [tok, feat] (tiles [128, 512] x16)
  - scores computed transposed: S^T[k, q] = kTw_chunk.T-stationary @ qT
    (2 heads row-packed per matmul pair), softmax needs no max subtraction
    (|scores| < ~2 for this problem; exp cannot overflow)
  - P~ = exp(S^T) with ACT; per-key weights already in kTw
  - PV: out^T[d, q] += v_chunk-stationary @ P~_chunk, 2 heads col-packed
  - softmax denominators = ones-vector matmul over accumulated P~ partials,
    reciprocal on DVE, partition-broadcast via SWDGE DMA, applied during the
    PSUM eviction of out^T
  - final: y[tok, out] += attT_chunk-stationary @ Wo, evict + DMA
"""

import time
from contextlib import ExitStack
from dataclasses import dataclass

import numpy as np

import concourse.bass as bass
import concourse.tile as tile
from concourse import bacc, mybir
from concourse.bass_utils import run_bass_kernel_spmd

F32 = mybir.dt.float32
F32R = mybir.dt.float32r
BF16 = mybir.dt.bfloat16
EXP = mybir.ActivationFunctionType.Exp

N_CORES = 8
NUM_HEADS = 16
HD = 64


@dataclass(frozen=True)
class Cfg:
    D: int = 1024    # input embedding dim
    T: int = 2048    # tokens (Nq == Nk)
    F: int = 512     # projected feats per core (8 heads * 64)
    OUT: int = 1024  # Wo output dim
    QTW: int = 512   # q-tile width for attention

    @property
    def IC(self):    # input-dim chunks of 128
        return self.D // 128

    @property
    def KC(self):    # key chunks of 128
        return self.T // 128

    @property
    def NF(self):    # feat tiles of 128 (head pairs)
        return self.F // 128

    @property
    def NQT(self):   # q tiles
        return self.T // self.QTW

    @property
    def PW(self):    # projection pass width (token cols per psum pass)
        return min(self.T, 1024)

    @property
    def NPASS(self):
        return self.T // self.PW


FULL = Cfg()


def emit(ctx: ExitStack, tc, io: dict, cfg: Cfg, blevel: int = 4, do_c: bool = True):
    nc = tc.nc
    D, T, F, OUT, QTW = cfg.D, cfg.T, cfg.F, cfg.OUT, cfg.QTW
    IC, KC, NF, NQT = cfg.IC, cfg.KC, cfg.NF, cfg.NQT
    PJW = min(T, 512)        # projection psum pass width
    NPJ = T // PJW
    NO = max(OUT // 512, 1)  # out-dim chunks for final projection
    OCW = OUT // NO
    assert QTW <= 512 and OCW <= 512
    SPLIT_ADDS = False  # gpsimd TT too slow; congests bcast DMAs

    # ---------------- persistent pools / tiles ----------------
    qkv_pool = ctx.enter_context(tc.tile_pool(name="qkv", bufs=1))
    const_pool = ctx.enter_context(tc.tile_pool(name="const", bufs=1))
    x_pool = ctx.enter_context(tc.tile_pool(name="xchunk", bufs=min(IC, 8) + 4))
    w_pool = ctx.enter_context(tc.tile_pool(name="wproj", bufs=3))
    wv_pool = ctx.enter_context(tc.tile_pool(name="wvp", bufs=1))
    wkb_pool = ctx.enter_context(tc.tile_pool(name="wkbp", bufs=1))
    pj_psum = ctx.enter_context(tc.tile_pool(name="pjpsum", bufs=1, space="PSUM"))

    ones = const_pool.tile([128, 1], BF16, tag="ones")
    nc.vector.memset(ones, 1.0)

    qT = [qkv_pool.tile([128, T], BF16, tag=f"qT{f}", name=f"qT{f}") for f in range(NF)]
    kTw = [qkv_pool.tile([128, T], BF16, tag=f"kTw{f}", name=f"kTw{f}") for f in range(NF)]
    vtok = [qkv_pool.tile([128, F], BF16, tag=f"v{t}", name=f"v{t}") for t in range(KC)]

    # weight_K broadcast to all 128 partitions: [128, T]
    wkb = wkb_pool.tile([128, T], F32, tag="wkb")
    wk_ap = io["wkey"]
    wk_src = bass.AP(tensor=wk_ap.tensor, offset=wk_ap.offset,
                     ap=[[0, 128]] + list(wk_ap.ap))
    nc.gpsimd.dma_start(out=wkb[:], in_=wk_src)

    def load_x(name):
        # hold one full transposed input in SBUF as IC slabs [128, T]
        xs = []
        for i in range(IC):
            xt = x_pool.tile([128, T], BF16, tag="xchunk", name=f"x{name}{i}")
            nc.sync.dma_start(xt[:], io[name][i * 128:(i + 1) * 128, :])
            xs.append(xt)
        return xs

    # ------- q/k projections, emitted in half-passes so the PE detours are
    # ------- small enough to hide under the exp stream of the previous
    # ------- head-pair's attention loop -------
    def make_proj_passes(f, psum_pool, psum_tag):
        state = {}

        def prefetch(xname, wname, kind):
            def go():
                xs = load_x(xname)
                wt = w_pool.tile([128, IC, 128], BF16, tag="wproj",
                                 name=f"w{wname}{f}")
                nc.sync.dma_start(
                    wt[:],
                    io[wname][:, f * 128:(f + 1) * 128]
                    .rearrange("(i p) f -> p i f", p=128))
                state[kind] = (xs, wt)
            return go

        NP = 2 if IC >= 2 else 1  # pieces per pass (4 matmuls each at IC=8)

        def mk(dst, kind, pa, piece):
            def go():
                xs, wt = state[kind]
                psl = slice(pa * PJW, (pa + 1) * PJW)
                if piece == 0:
                    state["ps"] = psum_pool.tile([128, PJW], F32, tag=psum_tag,
                                                 name=f"pj{kind}{f}{pa}")
                ps = state["ps"]
                for i in range(piece * IC // NP, (piece + 1) * IC // NP):
                    nc.tensor.matmul(ps[:], wt[:, i, :], xs[i][:, psl],
                                     start=(i == 0), stop=(i == IC - 1))
                if piece == NP - 1:
                    if kind == "q":
                        nc.vector.tensor_copy(dst[:, psl], ps[:])
                    else:  # k: fold per-key weight during eviction
                        nc.vector.tensor_mul(dst[:, psl], ps[:], wkb[:, psl])
            return go

        halves = ([mk(qT[f], "q", pa, p) for pa in range(NPJ) for p in range(NP)]
                  + [mk(kTw[f], "k", pa, p) for pa in range(NPJ) for p in range(NP)])
        return prefetch("xq", "wq", "q"), prefetch("xk", "wk", "k"), halves

    # ---------------- stages B + C ----------------
    with ExitStack() as sb:
        att_pool = sb.enter_context(tc.tile_pool(name="attp", bufs=1))
        wo_pool = sb.enter_context(tc.tile_pool(name="wop", bufs=1))
        pexp_pool = sb.enter_context(tc.tile_pool(name="pexp", bufs=10))
        part_pool = sb.enter_context(tc.tile_pool(name="partials", bufs=6))
        inv_pool = sb.enter_context(tc.tile_pool(name="invp", bufs=2))
        scr_pool = sb.enter_context(tc.tile_pool(name="scrp", bufs=2, space="DRAM"))

        s_psum = sb.enter_context(tc.tile_pool(name="spsum", bufs=2, space="PSUM"))
        o_psum = sb.enter_context(tc.tile_pool(name="opsum", bufs=2, space="PSUM"))
        sm_psum = sb.enter_context(tc.tile_pool(name="smpsum", bufs=1, space="PSUM"))

        attT = [att_pool.tile([128, T], BF16, tag=f"attT{f}", name=f"attT{f}") for f in range(NF)]

        wo_sb = wo_pool.tile([128, F // 128, OUT], BF16, tag="wo")
        nc.sync.dma_start(
            wo_sb[:], io["wo"].rearrange("(i p) o -> p i o", p=128))

        # ---------------- v projection (token-major) ----------------
        # runs in the attention score psum slots (idle during the prelude)
        xs = load_x("xv")
        wv_sb = wv_pool.tile([128, IC, F], BF16, tag="wv")
        nc.sync.dma_start(wv_sb[:], io["wv"].rearrange("(i p) f -> p i f", p=128))
        for t in range(KC):
            ps = s_psum.tile([128, PJW], F32, tag="s", name=f"vps{t}")
            for i in range(IC):
                nc.tensor.matmul(ps[:, 0:F], xs[i][:, t * 128:(t + 1) * 128],
                                 wv_sb[:, i, 0:F],
                                 start=(i == 0), stop=(i == IC - 1))
            nc.vector.tensor_copy(vtok[t][:], ps[:, 0:F])

        def emit_pv(hp, c, pe_t, o_ps):
            # out^T accumulation, 2 heads col-packed into one bank
            nc.tensor.matmul(o_ps[0:64, :],
                             vtok[c][:, hp * 128:hp * 128 + 64],
                             pe_t[:, 0:QTW],
                             start=(c == 0), stop=(c == KC - 1),
                             skip_group_check=True)
            nc.tensor.matmul(o_ps[64:128, :],
                             vtok[c][:, hp * 128 + 64:hp * 128 + 128],
                             pe_t[:, QTW:2 * QTW],
                             start=(c == 0), stop=(c == KC - 1),
                             skip_group_check=True)

        def norm_tail_a1(hp, qt, o_ps, pab_t, pab2_t):
            # head-A denominator: one sums matmul + one reciprocal
            two = SPLIT_ADDS
            sm_ps = sm_psum.tile([128, QTW], F32, tag="sm")
            nc.tensor.matmul(sm_ps[0:1, :], ones[:], pab_t[:, 0:QTW],
                             start=True, stop=not two, skip_group_check=True)
            if two:
                nc.tensor.matmul(sm_ps[0:1, :], ones[:], pab2_t[:, 0:QTW],
                                 start=False, stop=True, skip_group_check=True)
            inv1 = inv_pool.tile([1, 2 * QTW], F32, tag="inv1")
            nc.vector.reciprocal(inv1[:, 0:QTW], sm_ps[0:1, :])
            return sm_ps, inv1

        def norm_tail_a2(hp, qt, o_ps, pab_t, pab2_t, sm_ps, inv1):
            # head-B denominator + start the partition-broadcast (DRAM
            # roundtrip; SBUF APs cannot have zero-step partition dims)
            two = SPLIT_ADDS
            nc.tensor.matmul(sm_ps[32:33, :], ones[:], pab_t[:, QTW:2 * QTW],
                             start=True, stop=not two, skip_group_check=True)
            if two:
                nc.tensor.matmul(sm_ps[32:33, :], ones[:],
                                 pab2_t[:, QTW:2 * QTW],
                                 start=False, stop=True, skip_group_check=True)
            nc.vector.reciprocal(inv1[:, QTW:2 * QTW], sm_ps[32:33, :])
            scr = scr_pool.tile([1, 2 * QTW], F32, tag="scr")
            nc.gpsimd.dma_start(scr[:], inv1[:])
            invb = inv_pool.tile([128, QTW], F32, tag="invb")
            scr_a = scr[:, 0:QTW]
            scr_b = scr[:, QTW:2 * QTW]
            nc.gpsimd.dma_start(
                out=invb[0:64, :],
                in_=bass.AP(tensor=scr_a.tensor, offset=scr_a.offset,
                            ap=[[0, 64], [1, QTW]]))
            nc.gpsimd.dma_start(
                out=invb[64:128, :],
                in_=bass.AP(tensor=scr_b.tensor, offset=scr_b.offset,
                            ap=[[0, 64], [1, QTW]]))
            return invb

        def norm_tail_b(hp, qt, o_ps, invb, half):
            # normalize + evict out^T into attT, one head per call (small DVE
            # detours keep the partial-add chain at the exp rate)
            qsl = slice(qt * QTW, (qt + 1) * QTW)
            rs = slice(0, 64) if half == 0 else slice(64, 128)
            nc.vector.tensor_mul(attT[hp][rs, qsl], o_ps[rs, :], invb[rs, :])

        y_pool = sb.enter_context(tc.tile_pool(name="yev", bufs=2))
        y_sbs = {}
        y_state = [0]

        def emit_y_half(t, n):
            # half an output-projection token tile; psum via the (idle) pj
            # pool, DVE eviction to keep ACT free for exp
            tsl = slice(t * 128, (t + 1) * 128)
            if n == 0:
                y_sbs[t] = y_pool.tile([128, OUT], F32, tag="ysb",
                                       name=f"ysb{t}")
            y_sb = y_sbs[t]
            y_ps = pj_psum.tile([128, OCW], F32, tag="pj", name=f"yps{t}_{n}")
            for f in range(NF):
                nc.tensor.matmul(
                    y_ps[:],
                    attT[f][:, tsl],
                    wo_sb[:, f, n * OCW:(n + 1) * OCW],
                    start=(f == 0), stop=(f == NF - 1))
            nc.vector.tensor_copy(y_sb[:, n * OCW:(n + 1) * OCW], y_ps[:])
            if n == NO - 1:
                nc.sync.dma_start(io["y"][tsl, :], y_sb[:])

        TA = min(2, KC - 1)   # chunk at which the previous tail's sums start
        TB = 11  # chunk at which the previous tail's mul runs
        pending = None
        pending_b = None

        def run_proj_serial(f, pool=None, tag=None):
            pq, pk, halves = make_proj_passes(f, pool or pj_psum, tag or "pj")
            pq()
            for h in halves[:len(halves) // 2]:
                h()
            pk()
            for h in halves[len(halves) // 2:]:
                h()

        run_proj_serial(0, o_psum, "o")
        if blevel == 0:
            for f in range(1, NF):
                run_proj_serial(f)
            return
        for hp in range(NF):          # head pair = feat tile
            if hp + 1 < NF:
                pf_q, pf_k, next_passes = make_proj_passes(hp + 1, pj_psum, "pj")
            else:
                pf_q = pf_k = None
                next_passes = []
            npi = 0
            for qt in range(NQT):
                qsl = slice(qt * QTW, (qt + 1) * QTW)
                o_ps = o_psum.tile([128, QTW], F32, tag="o")
                pab_t = part_pool.tile([128, 2 * QTW], BF16, tag="pab")
                pab2_t = (part_pool.tile([128, 2 * QTW], BF16, tag="pab2",
                                         name="pab2_t")
                          if SPLIT_ADDS else None)
                pv_queue = []
                for c in range(KC):
                    csl = slice(c * 128, (c + 1) * 128)
                    s_ps = s_psum.tile([128, 2 * QTW], F32, tag="s")
                    # S^T chunk, head A (rows 0:64) and B (64:128) row-packed
                    nc.tensor.matmul(s_ps[:, 0:QTW],
                                     kTw[hp][0:64, csl], qT[hp][0:64, qsl],
                                     start=True, stop=True)
                    nc.tensor.matmul(s_ps[:, QTW:2 * QTW],
                                     kTw[hp][64:128, csl], qT[hp][64:128, qsl],
                                     start=True, stop=True)
                    if blevel >= 2:
                        # P~ = exp(S^T), both heads in one ACT op
                        pe_t = pexp_pool.tile([128, 2 * QTW], BF16, tag="pe")
                        nc.scalar.activation(pe_t[:], s_ps[:], EXP)
                        # running partial sums (softmax denominators),
                        # split 3:1 across DVE and GPSIMD (independent
                        # accumulators, merged in the sums matmuls)
                        if blevel >= 4:
                            if SPLIT_ADDS and c % 4 == 3:
                                if c == 3:
                                    nc.gpsimd.tensor_copy(pab2_t[:], pe_t[:])
                                else:
                                    nc.gpsimd.tensor_add(pab2_t[:], pab2_t[:],
                                                         pe_t[:])
                            elif c == 0:
                                nc.vector.tensor_copy(pab_t[:], pe_t[:])
                            else:
                                nc.vector.tensor_add(pab_t[:], pab_t[:],
                                                     pe_t[:])
                        if blevel >= 3:
                            # PV deferred and emitted in pairs on odd chunks:
                            # the PE stream sees QK,QK,PV,PV per 2 chunks,
                            # halving the row-conflicting LDW transitions,
                            # and never stalls on exp with QK work queued
                            pv_queue.append((c, pe_t))
                            while len(pv_queue) > 1:
                                pc, ppe = pv_queue.pop(0)
                                emit_pv(hp, pc, ppe, o_ps)
                    if pf_q is not None and qt == 0 and c == 0:
                        pf_q()
                    if pf_k is not None and qt == 1 and c == 8:
                        pf_k()
                    if KC >= 8 and c == TA and pending is not None:
                        st = norm_tail_a1(*pending)
                        pending = pending + st
                    if KC >= 8 and c == TA + 4 and pending is not None:
                        invb = norm_tail_a2(*pending)
                        pending_b = (pending[0], pending[1], pending[2], invb)
                        pending = None
                    if KC >= 8 and c == TB and pending_b is not None:
                        norm_tail_b(*pending_b, 0)
                    if KC >= 8 and c == TB + 2 and pending_b is not None:
                        norm_tail_b(*pending_b, 1)
                        pending_b = None
                    slots = (8, 12, 15) if qt == 0 else (4, 8, 12, 15)
                    if c in slots and npi < len(next_passes):
                        next_passes[npi]()
                        npi += 1
                    if do_c and blevel >= 4 and hp == NF - 1:
                        ready_t = ((qt - 1) if c >= TB + 1 else (qt - 2)) \
                            * (QTW // 128)
                        slots_y = ((11, 13, 15) if qt == 1
                                   else (1, 3, 5, 7, 9, 11, 13, 15))
                        if (qt >= 1 and c in slots_y
                                and y_state[0] < NO * max(ready_t, 0)):
                            emit_y_half(y_state[0] // NO, y_state[0] % NO)
                            y_state[0] += 1
                for pc, ppe in pv_queue:
                    emit_pv(hp, pc, ppe, o_ps)
                pv_queue = []
                if blevel >= 4:
                    if KC < 8 and pending is not None:
                        # small-KC fallback: run the whole tail serially
                        st = norm_tail_a1(*pending)
                        invb = norm_tail_a2(*(pending + st))
                        norm_tail_b(pending[0], pending[1], pending[2], invb, 0)
                        norm_tail_b(pending[0], pending[1], pending[2], invb, 1)
                    pending = (hp, qt, o_ps, pab_t, pab2_t)
            # any projection passes not yet injected (shouldn't happen for
            # NQT*2 >= 2*NPJ, but keep correct regardless)
            while npi < len(next_passes):
                next_passes[npi]()
                npi += 1
        if pending is not None:
            if len(pending) == 5:
                pending = pending + norm_tail_a1(*pending)
            invb = norm_tail_a2(*pending)
            pending_b = (pending[0], pending[1], pending[2], invb)
        if pending_b is not None:
            norm_tail_b(*pending_b, 0)
            norm_tail_b(*pending_b, 1)

        if not do_c or blevel < 4:
            return
        # ---------------- stage C: remaining output tiles ----------------
        while y_state[0] < NO * KC:
            emit_y_half(y_state[0] // NO, y_state[0] % NO)
            y_state[0] += 1


def build_program(cfg: Cfg = FULL, n_iters: int = 1, blevel: int = 4, do_c: bool = True):
    nc = bacc.Bacc("TRN2", target_bir_lowering=False, debug=False,
                   num_devices=N_CORES)
    io = {
        "xq": nc.dram_tensor("xq", [cfg.D, cfg.T], BF16, kind="ExternalInput").ap(),
        "xk": nc.dram_tensor("xk", [cfg.D, cfg.T], BF16, kind="ExternalInput").ap(),
        "xv": nc.dram_tensor("xv", [cfg.D, cfg.T], BF16, kind="ExternalInput").ap(),
        "wq": nc.dram_tensor("wq", [cfg.D, cfg.F], BF16, kind="ExternalInput").ap(),
        "wk": nc.dram_tensor("wk", [cfg.D, cfg.F], BF16, kind="ExternalInput").ap(),
        "wv": nc.dram_tensor("wv", [cfg.D, cfg.F], BF16, kind="ExternalInput").ap(),
        "wo": nc.dram_tensor("wo", [cfg.F, cfg.OUT], BF16, kind="ExternalInput").ap(),
        "wkey": nc.dram_tensor("wkey", [cfg.T], F32, kind="ExternalInput").ap(),
        "y": nc.dram_tensor("y", [cfg.T, cfg.OUT], F32, kind="ExternalOutput").ap(),
    }
    with tile.TileContext(nc) as tc:
        with ExitStack() as ctx:
            if n_iters == 1:
                emit(ctx, tc, io, cfg, blevel, do_c)
            else:
                with tc.For_i(0, n_iters, 1):
                    emit(ctx, tc, io, cfg, blevel, do_c)
    nc.compile()
    return nc


def shard_inputs(Q_tokens, K_tokens, V_tokens, weight_K, Wq, Wk, Wv, Wo, bo):
    """Build the 8 per-core input maps (full -> sharded, host side)."""
    import ml_dtypes
    bf = ml_dtypes.bfloat16
    scale = 1.0 / np.sqrt(np.float32(HD))
    Wq_s = (np.asarray(Wq, np.float32) * scale)
    in_maps = []
    qt = np.ascontiguousarray(np.asarray(Q_tokens, np.float32).transpose(0, 2, 1)).astype(bf)
    kt = np.ascontiguousarray(np.asarray(K_tokens, np.float32).transpose(0, 2, 1)).astype(bf)
    vt = np.ascontiguousarray(np.asarray(V_tokens, np.float32).transpose(0, 2, 1)).astype(bf)
    for c in range(N_CORES):
        b, g = divmod(c, 2)
        fs = slice(g * 512, (g + 1) * 512)
        in_maps.append({
            "xq": qt[b],
            "xk": kt[b],
            "xv": vt[b],
            "wq": np.ascontiguousarray(Wq_s[:, fs]).astype(bf),
            "wk": np.ascontiguousarray(np.asarray(Wk, np.float32)[:, fs]).astype(bf),
            "wv": np.ascontiguousarray(np.asarray(Wv, np.float32)[:, fs]).astype(bf),
            "wo": np.ascontiguousarray(np.asarray(Wo, np.float32)[fs, :]).astype(bf),
            "wkey": np.ascontiguousarray(np.asarray(weight_K, np.float32)[b]),
        })
    return in_maps


_PROGRAM_CACHE: dict = {}


def _get_program(n_iters: int = 1, blevel: int = 4, do_c: bool = True):
    key = (n_iters, blevel, do_c)
    if key not in _PROGRAM_CACHE:
        _PROGRAM_CACHE[key] = build_program(FULL, n_iters, blevel, do_c)
    return _PROGRAM_CACHE[key]


def run_sharded(in_maps, n_iters: int = 1):
    nc = _get_program(n_iters)
    t0 = time.time()
    res = run_bass_kernel_spmd(nc, in_maps, core_ids=list(range(N_CORES)))
    wall = time.time() - t0
    return res, wall


def kernel(Q_tokens, K_tokens, V_tokens, weight_K, Wq, Wk, Wv, Wo, bo):
    in_maps = shard_inputs(Q_tokens, K_tokens, V_tokens, weight_K,
                           Wq, Wk, Wv, Wo, bo)
    res, _ = run_sharded(in_maps)
    B = np.asarray(Q_tokens).shape[0]
    T, OUT = FULL.T, FULL.OUT
    out = np.empty((B, T, OUT), np.float32)
    bo = np.asarray(bo, np.float32)
    for b in range(B):
        out[b] = res.results[2 * b]["y"] + res.results[2 * b + 1]["y"] + bo
    return out


# revision 47
# speedup vs baseline: 1.0698x; 1.0030x over previous
"""Trainium2 Bass kernel for nn_CrossAttentionFuse.

Reference computation (per batch b):
    q = Q_tokens[b] @ Wq ; k = K_tokens[b] @ Wk ; v = V_tokens[b] @ Wv   (all [T, 1024])
    per head h (16 heads x 64): kw = k_h * weight_K[b][:, None]
    S = q_h @ kw_h.T / sqrt(64) ; P = softmax(S) ; o_h = P @ v_h
    y[b] = concat_h(o_h) @ Wo + bo

Sharding (8 cores): core c handles batch b = c//2, head-group g = c%2 (8 heads,
512 feats).  Each core computes a partial y (its 8 heads' contribution to the
output projection); host sums the two partials per batch and adds bo.

Per-core layout strategy (all activations kept feature-major, "transposed"):
  - inputs are host-transposed X^T [1024, T]
  - qT, kTw [feat, tok] tiles ([128, T] x4; feat tile f = heads 2f, 2f+1)
  - kTw = k^T * weight_K (folded during PSUM eviction; removes softmax scale)
  - 1/sqrt(64) folded into Wq on host
  - v token-major [tok, feat] (tiles [128, 512] x16)
  - scores computed transposed: S^T[k, q] = kTw_chunk.T-stationary @ qT
    (2 heads row-packed per matmul pair), softmax needs no max subtraction
    (|scores| < ~2 for this problem; exp cannot overflow)
  - P~ = exp(S^T) with ACT; per-key weights already in kTw
  - PV: out^T[d, q] += v_chunk-stationary @ P~_chunk, 2 heads col-packed
  - softmax denominators = ones-vector matmul over accumulated P~ partials,
    reciprocal on DVE, partition-broadcast via SWDGE DMA, applied during the
    PSUM eviction of out^T
  - final: y[tok, out] += attT_chunk-stationary @ Wo, evict + DMA
"""

import time
from contextlib import ExitStack
from dataclasses import dataclass

import numpy as np

import concourse.bass as bass
import concourse.tile as tile
from concourse import bacc, mybir
from concourse.bass_utils import run_bass_kernel_spmd

F32 = mybir.dt.float32
F32R = mybir.dt.float32r
BF16 = mybir.dt.bfloat16
EXP = mybir.ActivationFunctionType.Exp

N_CORES = 8
NUM_HEADS = 16
HD = 64


@dataclass(frozen=True)
class Cfg:
    D: int = 1024    # input embedding dim
    T: int = 2048    # tokens (Nq == Nk)
    F: int = 512     # projected feats per core (8 heads * 64)
    OUT: int = 1024  # Wo output dim
    QTW: int = 512   # q-tile width for attention

    @property
    def IC(self):    # input-dim chunks of 128
        return self.D // 128

    @property
    def KC(self):    # key chunks of 128
        return self.T // 128

    @property
    def NF(self):    # feat tiles of 128 (head pairs)
        return self.F // 128

    @property
    def NQT(self):   # q tiles
        return self.T // self.QTW

    @property
    def PW(self):    # projection pass width (token cols per psum pass)
        return min(self.T, 1024)

    @property
    def NPASS(self):
        return self.T // self.PW


FULL = Cfg()


def emit(ctx: ExitStack, tc, io: dict, cfg: Cfg, blevel: int = 4, do_c: bool = True):
    nc = tc.nc
    D, T, F, OUT, QTW = cfg.D, cfg.T, cfg.F, cfg.OUT, cfg.QTW
    IC, KC, NF, NQT = cfg.IC, cfg.KC, cfg.NF, cfg.NQT
    PJW = min(T, 512)        # projection psum pass width
    NPJ = T // PJW
    NO = max(OUT // 512, 1)  # out-dim chunks for final projection
    OCW = OUT // NO
    assert QTW <= 512 and OCW <= 512
    SPLIT_ADDS = False  # gpsimd TT too slow; congests bcast DMAs

    # ---------------- persistent pools / tiles ----------------
    qkv_pool = ctx.enter_context(tc.tile_pool(name="qkv", bufs=1))
    const_pool = ctx.enter_context(tc.tile_pool(name="const", bufs=1))
    x_pool = ctx.enter_context(tc.tile_pool(name="xchunk", bufs=min(IC, 8) + 4))
    w_pool = ctx.enter_context(tc.tile_pool(name="wproj", bufs=3))
    wv_pool = ctx.enter_context(tc.tile_pool(name="wvp", bufs=1))
    wkb_pool = ctx.enter_context(tc.tile_pool(name="wkbp", bufs=1))
    pj_psum = ctx.enter_context(tc.tile_pool(name="pjpsum", bufs=1, space="PSUM"))

    ones = const_pool.tile([128, 1], BF16, tag="ones")
    nc.vector.memset(ones, 1.0)

    qT = [qkv_pool.tile([128, T], BF16, tag=f"qT{f}", name=f"qT{f}") for f in range(NF)]
    kTw = [qkv_pool.tile([128, T], BF16, tag=f"kTw{f}", name=f"kTw{f}") for f in range(NF)]
    vtok = [qkv_pool.tile([128, F], BF16, tag=f"v{t}", name=f"v{t}") for t in range(KC)]

    # weight_K broadcast to all 128 partitions: [128, T]
    wkb = wkb_pool.tile([128, T], F32, tag="wkb")
    wk_ap = io["wkey"]
    wk_src = bass.AP(tensor=wk_ap.tensor, offset=wk_ap.offset,
                     ap=[[0, 128]] + list(wk_ap.ap))
    nc.gpsimd.dma_start(out=wkb[:], in_=wk_src)

    def load_x(name):
        # hold one full transposed input in SBUF as IC slabs [128, T]
        xs = []
        for i in range(IC):
            xt = x_pool.tile([128, T], BF16, tag="xchunk", name=f"x{name}{i}")
            nc.sync.dma_start(xt[:], io[name][i * 128:(i + 1) * 128, :])
            xs.append(xt)
        return xs

    # ------- q/k projections, emitted in half-passes so the PE detours are
    # ------- small enough to hide under the exp stream of the previous
    # ------- head-pair's attention loop -------
    def make_proj_passes(f, psum_pool, psum_tag):
        state = {}

        def prefetch(xname, wname, kind):
            def go():
                xs = load_x(xname)
                wt = w_pool.tile([128, IC, 128], BF16, tag="wproj",
                                 name=f"w{wname}{f}")
                nc.sync.dma_start(
                    wt[:],
                    io[wname][:, f * 128:(f + 1) * 128]
                    .rearrange("(i p) f -> p i f", p=128))
                state[kind] = (xs, wt)
            return go

        NP = 2 if IC >= 2 else 1  # pieces per pass (4 matmuls each at IC=8)

        def mk(dst, kind, pa, piece):
            def go():
                xs, wt = state[kind]
                psl = slice(pa * PJW, (pa + 1) * PJW)
                if piece == 0:
                    state["ps"] = psum_pool.tile([128, PJW], F32, tag=psum_tag,
                                                 name=f"pj{kind}{f}{pa}")
                ps = state["ps"]
                for i in range(piece * IC // NP, (piece + 1) * IC // NP):
                    nc.tensor.matmul(ps[:], wt[:, i, :], xs[i][:, psl],
                                     start=(i == 0), stop=(i == IC - 1))
                if piece == NP - 1:
                    if kind == "q":
                        nc.vector.tensor_copy(dst[:, psl], ps[:])
                    else:  # k: fold per-key weight during eviction
                        nc.vector.tensor_mul(dst[:, psl], ps[:], wkb[:, psl])
            return go

        halves = ([mk(qT[f], "q", pa, p) for pa in range(NPJ) for p in range(NP)]
                  + [mk(kTw[f], "k", pa, p) for pa in range(NPJ) for p in range(NP)])
        return prefetch("xq", "wq", "q"), prefetch("xk", "wk", "k"), halves

    # ---------------- stages B + C ----------------
    with ExitStack() as sb:
        att_pool = sb.enter_context(tc.tile_pool(name="attp", bufs=1))
        wo_pool = sb.enter_context(tc.tile_pool(name="wop", bufs=1))
        pexp_pool = sb.enter_context(tc.tile_pool(name="pexp", bufs=10))
        part_pool = sb.enter_context(tc.tile_pool(name="partials", bufs=6))
        inv_pool = sb.enter_context(tc.tile_pool(name="invp", bufs=2))
        scr_pool = sb.enter_context(tc.tile_pool(name="scrp", bufs=2, space="DRAM"))

        s_psum = sb.enter_context(tc.tile_pool(name="spsum", bufs=2, space="PSUM"))
        o_psum = sb.enter_context(tc.tile_pool(name="opsum", bufs=2, space="PSUM"))
        sm_psum = sb.enter_context(tc.tile_pool(name="smpsum", bufs=1, space="PSUM"))

        attT = [att_pool.tile([128, T], BF16, tag=f"attT{f}", name=f"attT{f}") for f in range(NF)]

        wo_sb = wo_pool.tile([128, F // 128, OUT], BF16, tag="wo")
        nc.sync.dma_start(
            wo_sb[:], io["wo"].rearrange("(i p) o -> p i o", p=128))

        # ---------------- v projection (token-major) ----------------
        # runs in the attention score psum slots (idle during the prelude)
        xs = load_x("xv")
        wv_sb = wv_pool.tile([128, IC, F], BF16, tag="wv")
        nc.sync.dma_start(wv_sb[:], io["wv"].rearrange("(i p) f -> p i f", p=128))
        for t in range(KC):
            ps = s_psum.tile([128, PJW], F32, tag="s", name=f"vps{t}")
            for i in range(IC):
                nc.tensor.matmul(ps[:, 0:F], xs[i][:, t * 128:(t + 1) * 128],
                                 wv_sb[:, i, 0:F],
                                 start=(i == 0), stop=(i == IC - 1))
            nc.vector.tensor_copy(vtok[t][:], ps[:, 0:F])

        def emit_pv(hp, c, pe_t, o_ps):
            # out^T accumulation, 2 heads col-packed into one bank
            nc.tensor.matmul(o_ps[0:64, :],
                             vtok[c][:, hp * 128:hp * 128 + 64],
                             pe_t[:, 0:QTW],
                             start=(c == 0), stop=(c == KC - 1),
                             skip_group_check=True)
            nc.tensor.matmul(o_ps[64:128, :],
                             vtok[c][:, hp * 128 + 64:hp * 128 + 128],
                             pe_t[:, QTW:2 * QTW],
                             start=(c == 0), stop=(c == KC - 1),
                             skip_group_check=True)

        def norm_tail_a1(hp, qt, o_ps, pab_t, pab2_t):
            # head-A denominator: one sums matmul + one reciprocal
            two = SPLIT_ADDS
            sm_ps = sm_psum.tile([128, QTW], F32, tag="sm")
            nc.tensor.matmul(sm_ps[0:1, :], ones[:], pab_t[:, 0:QTW],
                             start=True, stop=not two, skip_group_check=True)
            if two:
                nc.tensor.matmul(sm_ps[0:1, :], ones[:], pab2_t[:, 0:QTW],
                                 start=False, stop=True, skip_group_check=True)
            inv1 = inv_pool.tile([1, 2 * QTW], F32, tag="inv1")
            nc.vector.reciprocal(inv1[:, 0:QTW], sm_ps[0:1, :])
            return sm_ps, inv1

        def norm_tail_a2(hp, qt, o_ps, pab_t, pab2_t, sm_ps, inv1):
            # head-B denominator + start the partition-broadcast (DRAM
            # roundtrip; SBUF APs cannot have zero-step partition dims)
            two = SPLIT_ADDS
            nc.tensor.matmul(sm_ps[32:33, :], ones[:], pab_t[:, QTW:2 * QTW],
                             start=True, stop=not two, skip_group_check=True)
            if two:
                nc.tensor.matmul(sm_ps[32:33, :], ones[:],
                                 pab2_t[:, QTW:2 * QTW],
                                 start=False, stop=True, skip_group_check=True)
            nc.vector.reciprocal(inv1[:, QTW:2 * QTW], sm_ps[32:33, :])
            scr = scr_pool.tile([1, 2 * QTW], F32, tag="scr")
            nc.gpsimd.dma_start(scr[:], inv1[:])
            invb = inv_pool.tile([128, QTW], F32, tag="invb")
            scr_a = scr[:, 0:QTW]
            scr_b = scr[:, QTW:2 * QTW]
            nc.gpsimd.dma_start(
                out=invb[0:64, :],
                in_=bass.AP(tensor=scr_a.tensor, offset=scr_a.offset,
                            ap=[[0, 64], [1, QTW]]))
            nc.gpsimd.dma_start(
                out=invb[64:128, :],
                in_=bass.AP(tensor=scr_b.tensor, offset=scr_b.offset,
                            ap=[[0, 64], [1, QTW]]))
            return invb

        def norm_tail_b(hp, qt, o_ps, invb, half):
            # normalize + evict out^T into attT, one head per call (small DVE
            # detours keep the partial-add chain at the exp rate)
            qsl = slice(qt * QTW, (qt + 1) * QTW)
            rs = slice(0, 64) if half == 0 else slice(64, 128)
            nc.vector.tensor_mul(attT[hp][rs, qsl], o_ps[rs, :], invb[rs, :])

        y_pool = sb.enter_context(tc.tile_pool(name="yev", bufs=2))
        y_sbs = {}
        y_state = [0]

        def emit_y_half(t, n):
            # half an output-projection token tile; psum via the (idle) pj
            # pool, DVE eviction to keep ACT free for exp
            tsl = slice(t * 128, (t + 1) * 128)
            if n == 0:
                y_sbs[t] = y_pool.tile([128, OUT], F32, tag="ysb",
                                       name=f"ysb{t}")
            y_sb = y_sbs[t]
            y_ps = pj_psum.tile([128, OCW], F32, tag="pj", name=f"yps{t}_{n}")
            for f in range(NF):
                nc.tensor.matmul(
                    y_ps[:],
                    attT[f][:, tsl],
                    wo_sb[:, f, n * OCW:(n + 1) * OCW],
                    start=(f == 0), stop=(f == NF - 1))
            nc.vector.tensor_copy(y_sb[:, n * OCW:(n + 1) * OCW], y_ps[:])
            if n == NO - 1:
                nc.sync.dma_start(io["y"][tsl, :], y_sb[:])

        TA = min(2, KC - 1)   # chunk at which the previous tail's sums start
        TB = 11  # chunk at which the previous tail's mul runs
        pending = None
        pending_b = None
        pv_queue = []

        def run_proj_serial(f, pool=None, tag=None):
            pq, pk, halves = make_proj_passes(f, pool or pj_psum, tag or "pj")
            pq()
            for h in halves[:len(halves) // 2]:
                h()
            pk()
            for h in halves[len(halves) // 2:]:
                h()

        run_proj_serial(0, o_psum, "o")
        if blevel == 0:
            for f in range(1, NF):
                run_proj_serial(f)
            return
        for hp in range(NF):          # head pair = feat tile
            if hp + 1 < NF:
                pf_q, pf_k, next_passes = make_proj_passes(hp + 1, pj_psum, "pj")
            else:
                pf_q = pf_k = None
                next_passes = []
            npi = 0
            for qt in range(NQT):
                qsl = slice(qt * QTW, (qt + 1) * QTW)
                o_ps = o_psum.tile([128, QTW], F32, tag="o")
                pab_t = part_pool.tile([128, 2 * QTW], BF16, tag="pab")
                pab2_t = (part_pool.tile([128, 2 * QTW], BF16, tag="pab2",
                                         name="pab2_t")
                          if SPLIT_ADDS else None)
                for c in range(KC):
                    csl = slice(c * 128, (c + 1) * 128)
                    s_ps = s_psum.tile([128, 2 * QTW], F32, tag="s")
                    # S^T chunk, head A (rows 0:64) and B (64:128) row-packed
                    nc.tensor.matmul(s_ps[:, 0:QTW],
                                     kTw[hp][0:64, csl], qT[hp][0:64, qsl],
                                     start=True, stop=True)
                    nc.tensor.matmul(s_ps[:, QTW:2 * QTW],
                                     kTw[hp][64:128, csl], qT[hp][64:128, qsl],
                                     start=True, stop=True)
                    if blevel >= 2:
                        # P~ = exp(S^T), both heads in one ACT op
                        pe_t = pexp_pool.tile([128, 2 * QTW], BF16, tag="pe")
                        nc.scalar.activation(pe_t[:], s_ps[:], EXP)
                        # running partial sums (softmax denominators),
                        # split 3:1 across DVE and GPSIMD (independent
                        # accumulators, merged in the sums matmuls)
                        if blevel >= 4:
                            if SPLIT_ADDS and c % 4 == 3:
                                if c == 3:
                                    nc.gpsimd.tensor_copy(pab2_t[:], pe_t[:])
                                else:
                                    nc.gpsimd.tensor_add(pab2_t[:], pab2_t[:],
                                                         pe_t[:])
                            elif c == 0:
                                nc.vector.tensor_copy(pab_t[:], pe_t[:])
                            else:
                                nc.vector.tensor_add(pab_t[:], pab_t[:],
                                                     pe_t[:])
                        if blevel >= 3:
                            # PV deferred and emitted in pairs on odd chunks:
                            # the PE stream sees QK,QK,PV,PV per 2 chunks,
                            # halving the row-conflicting LDW transitions,
                            # and never stalls on exp with QK work queued
                            pv_queue.append((hp, c, pe_t, o_ps))
                            while len(pv_queue) > 2:
                                php, pc, ppe, po = pv_queue.pop(0)
                                emit_pv(php, pc, ppe, po)
                    if pf_q is not None and qt == 0 and c == 0:
                        pf_q()
                    if pf_k is not None and qt == 1 and c == 8:
                        pf_k()
                    if KC >= 8 and c == TA and pending is not None:
                        st = norm_tail_a1(*pending)
                        pending = pending + st
                    if KC >= 8 and c == TA + 4 and pending is not None:
                        invb = norm_tail_a2(*pending)
                        pending_b = (pending[0], pending[1], pending[2], invb)
                        pending = None
                    if KC >= 8 and c == TB and pending_b is not None:
                        norm_tail_b(*pending_b, 0)
                    if KC >= 8 and c == TB + 2 and pending_b is not None:
                        norm_tail_b(*pending_b, 1)
                        pending_b = None
                    slots = (8, 12, 15) if qt == 0 else (4, 8, 12, 15)
                    if c in slots and npi < len(next_passes):
                        next_passes[npi]()
                        npi += 1
                    if do_c and blevel >= 4 and hp == NF - 1:
                        ready_t = ((qt - 1) if c >= TB + 1 else (qt - 2)) \
                            * (QTW // 128)
                        slots_y = ((10, 14, 15) if qt == 1
                                   else (1, 3, 5, 7, 9, 10, 14, 15))
                        if (qt >= 1 and c in slots_y
                                and y_state[0] < NO * max(ready_t, 0)):
                            emit_y_half(y_state[0] // NO, y_state[0] % NO)
                            y_state[0] += 1
                if blevel >= 4:
                    if KC < 8 and pending is not None:
                        # small-KC fallback: run the whole tail serially
                        st = norm_tail_a1(*pending)
                        invb = norm_tail_a2(*(pending + st))
                        norm_tail_b(pending[0], pending[1], pending[2], invb, 0)
                        norm_tail_b(pending[0], pending[1], pending[2], invb, 1)
                    pending = (hp, qt, o_ps, pab_t, pab2_t)
            # any projection passes not yet injected (shouldn't happen for
            # NQT*2 >= 2*NPJ, but keep correct regardless)
            while npi < len(next_passes):
                next_passes[npi]()
                npi += 1
        for php, pc, ppe, po in pv_queue:
            emit_pv(php, pc, ppe, po)
        pv_queue = []
        if pending is not None:
            if len(pending) == 5:
                pending = pending + norm_tail_a1(*pending)
            invb = norm_tail_a2(*pending)
            pending_b = (pending[0], pending[1], pending[2], invb)
        if pending_b is not None:
            norm_tail_b(*pending_b, 0)
            norm_tail_b(*pending_b, 1)

        if not do_c or blevel < 4:
            return
        # ---------------- stage C: remaining output tiles ----------------
        while y_state[0] < NO * KC:
            emit_y_half(y_state[0] // NO, y_state[0] % NO)
            y_state[0] += 1


def build_program(cfg: Cfg = FULL, n_iters: int = 1, blevel: int = 4, do_c: bool = True):
    nc = bacc.Bacc("TRN2", target_bir_lowering=False, debug=False,
                   num_devices=N_CORES)
    io = {
        "xq": nc.dram_tensor("xq", [cfg.D, cfg.T], BF16, kind="ExternalInput").ap(),
        "xk": nc.dram_tensor("xk", [cfg.D, cfg.T], BF16, kind="ExternalInput").ap(),
        "xv": nc.dram_tensor("xv", [cfg.D, cfg.T], BF16, kind="ExternalInput").ap(),
        "wq": nc.dram_tensor("wq", [cfg.D, cfg.F], BF16, kind="ExternalInput").ap(),
        "wk": nc.dram_tensor("wk", [cfg.D, cfg.F], BF16, kind="ExternalInput").ap(),
        "wv": nc.dram_tensor("wv", [cfg.D, cfg.F], BF16, kind="ExternalInput").ap(),
        "wo": nc.dram_tensor("wo", [cfg.F, cfg.OUT], BF16, kind="ExternalInput").ap(),
        "wkey": nc.dram_tensor("wkey", [cfg.T], F32, kind="ExternalInput").ap(),
        "y": nc.dram_tensor("y", [cfg.T, cfg.OUT], F32, kind="ExternalOutput").ap(),
    }
    with tile.TileContext(nc) as tc:
        with ExitStack() as ctx:
            if n_iters == 1:
                emit(ctx, tc, io, cfg, blevel, do_c)
            else:
                with tc.For_i(0, n_iters, 1):
                    emit(ctx, tc, io, cfg, blevel, do_c)
    nc.compile()
    return nc


def shard_inputs(Q_tokens, K_tokens, V_tokens, weight_K, Wq, Wk, Wv, Wo, bo):
    """Build the 8 per-core input maps (full -> sharded, host side)."""
    import ml_dtypes
    bf = ml_dtypes.bfloat16
    scale = 1.0 / np.sqrt(np.float32(HD))
    Wq_s = (np.asarray(Wq, np.float32) * scale)
    in_maps = []
    qt = np.ascontiguousarray(np.asarray(Q_tokens, np.float32).transpose(0, 2, 1)).astype(bf)
    kt = np.ascontiguousarray(np.asarray(K_tokens, np.float32).transpose(0, 2, 1)).astype(bf)
    vt = np.ascontiguousarray(np.asarray(V_tokens, np.float32).transpose(0, 2, 1)).astype(bf)
    for c in range(N_CORES):
        b, g = divmod(c, 2)
        fs = slice(g * 512, (g + 1) * 512)
        in_maps.append({
            "xq": qt[b],
            "xk": kt[b],
            "xv": vt[b],
            "wq": np.ascontiguousarray(Wq_s[:, fs]).astype(bf),
            "wk": np.ascontiguousarray(np.asarray(Wk, np.float32)[:, fs]).astype(bf),
            "wv": np.ascontiguousarray(np.asarray(Wv, np.float32)[:, fs]).astype(bf),
            "wo": np.ascontiguousarray(np.asarray(Wo, np.float32)[fs, :]).astype(bf),
            "wkey": np.ascontiguousarray(np.asarray(weight_K, np.float32)[b]),
        })
    return in_maps


_PROGRAM_CACHE: dict = {}


def _get_program(n_iters: int = 1, blevel: int = 4, do_c: bool = True):
    key = (n_iters, blevel, do_c)
    if key not in _PROGRAM_CACHE:
        _PROGRAM_CACHE[key] = build_program(FULL, n_iters, blevel, do_c)
    return _PROGRAM_CACHE[key]


def run_sharded(in_maps, n_iters: int = 1):
    nc = _get_program(n_iters)
    t0 = time.time()
    res = run_bass_kernel_spmd(nc, in_maps, core_ids=list(range(N_CORES)))
    wall = time.time() - t0
    return res, wall


def kernel(Q_tokens, K_tokens, V_tokens, weight_K, Wq, Wk, Wv, Wo, bo):
    in_maps = shard_inputs(Q_tokens, K_tokens, V_tokens, weight_K,
                           Wq, Wk, Wv, Wo, bo)
    res, _ = run_sharded(in_maps)
    B = np.asarray(Q_tokens).shape[0]
    T, OUT = FULL.T, FULL.OUT
    out = np.empty((B, T, OUT), np.float32)
    bo = np.asarray(bo, np.float32)
    for b in range(B):
        out[b] = res.results[2 * b]["y"] + res.results[2 * b + 1]["y"] + bo
    return out
